# revision 1
# baseline (speedup 1.0000x reference)
"""Trainium2 Bass kernel for nn_Attention_326417514823.

Per-batch computation (B=8, N=2048, D=256), one batch per NeuronCore:
    S = Q @ K.T / sqrt(D)                  (N x N)
    S[q, :] = -1e9 where mask[q] == 0      (row masking by query index)
    A = softmax(S, axis=0)                 (normalize over q, per column k)
    A[q, :] = 0 where mask[q] == 0
    O = A @ V                              (N x D)

Algebra used on device: the softmax normalizer c[k] = sum_q E[q,k] is
per-column, so it folds into V (W[k,:] = V[k,:] / c[k]) and O = E @ W with
E = exp(S/16) * mask[q].  No max-subtraction is needed: scores/16 stay in
[-7, 7], and the reference's masked entries are exp(-1e9 - max) == 0
exactly in fp32, which the mask-multiply reproduces exactly (zero).

Device layout (transposed so the softmax reduction runs along the free axis
and neither matmul needs an on-chip transpose):
    ST[k, q] = KT.T @ QT   (KT = K.T, QT = Q.T, d on partitions)
    E[k, q]  = exp(ST/16) * mask_bcast              (bf16)
    c[k]     = sum_q E[k, q]  (fused accum in the DVE mask multiply)
    W[k, :]  = V[k, :] * (1/c[k])                   (bf16)
    OT[d, q] = sum_k W[k,d] * E[k,q]  (PSUM accumulation over k-blocks)
Host transposes OT back to O.

Pipelining: PSUM = 8 banks. 4 banks hold the q<1024 half of OT's
accumulators for the WHOLE kernel, so half of matmul-2 interleaves into
phase 1 (lagging LAG k-blocks behind the softmax pipeline). The score
tiles double-buffer in the other 4 banks; once phase 1 ends those 4 banks
are reused for the q>=1024 accumulators, accumulated chain-per-bank so
each store overlaps the remaining chains. DMA emissions are ordered by
first consumption (HWDGE ring prep ~625 ns each is a shared serial
resource, and transfers serialize at ~360 GB/s).
"""

import numpy as np
import ml_dtypes

B, N, D = 8, 2048, 256
NCORES = 8
P = 128          # partitions
MMN = 512        # matmul moving free dim (one PSUM bank of fp32)
KB = N // P      # 16 k-blocks
NCH = N // MMN   # 4 512-chunks along q
DT = D // P      # 2 d-tiles
LAG = 4          # k-blocks of slack before interleaved matmul-2 consumes W
STT_SPLIT = False  # split mask-multiply per half: measured slower (DVE op overhead)

# "f32r": fp32 storage everywhere, TF32-class matmuls (1 cycle/row at
#         N>=256 per the TRN2 cost model) — most accurate (~3e-4).
# "mixed": Q/K in bf16 (halves the startup DMA-bus time; scores lose ~2e-3)
#         but E/W/c stay fp32r so the softmax/output path stays fp32-clean.
# "bf16": everything bf16 (~5e-3).
DTYPE_MODE = "f32r"

_cached = None


def _build():
    import concourse.bacc as bacc
    import concourse.mybir as mybir
    import concourse.tile as tile

    f32 = mybir.dt.float32
    bf16 = mybir.dt.bfloat16
    mmdt = bf16 if DTYPE_MODE == "bf16" else mybir.dt.float32r
    qkdt = mybir.dt.float32r if DTYPE_MODE == "f32r" else bf16
    MULT = mybir.AluOpType.mult
    EXP = mybir.ActivationFunctionType.Exp

    nc = bacc.Bacc()
    kt = nc.dram_tensor("kt", [D, N], qkdt, kind="ExternalInput")
    qt = nc.dram_tensor("qt", [D, N], qkdt, kind="ExternalInput")
    v = nc.dram_tensor("v", [N, D], f32, kind="ExternalInput")
    mb = nc.dram_tensor("mb", [1, N], bf16, kind="ExternalInput")
    ot = nc.dram_tensor("ot", [D, N], f32, kind="ExternalOutput")

    with tile.TileContext(nc) as tc:
        with (
            tc.tile_pool(name="const", bufs=1) as constp,
            tc.tile_pool(name="epool", bufs=1) as epool,
            tc.tile_pool(name="wpool", bufs=1) as wpool,
            tc.tile_pool(name="vpool", bufs=3) as vpool,
            tc.tile_pool(name="cpool", bufs=3) as cpool,
            tc.tile_pool(name="outp", bufs=6) as outp,
            # q<1024 OT accumulators live for the whole kernel (banks 0-3)
            tc.tile_pool(name="psA", bufs=1, space="PSUM") as psA,
        ):
            # inputs, chunked so the first matmuls start after ~128KB of DMA
            kt_ch = [[constp.tile([P, MMN], qkdt, name=f"ktc{d}_{j}")
                      for j in range(NCH)] for d in range(DT)]
            qt_ch = [[constp.tile([P, MMN], qkdt, name=f"qtc{d}_{j}")
                      for j in range(NCH)] for d in range(DT)]
            # DMA-ring choreography: kb=0 needs kt[*][0] and ALL qt chunks
            # immediately; kt[*][j] only at kb=4j; v at the k-block pace.
            # kt j0 goes on ScalarE's DGE ring (idle until the first exp) in
            # parallel with qt j0 on the SP ring; later kt chunks are emitted
            # inside the loop so v/mask don't queue behind them.
            def load_kt(d, j):
                nc.sync.dma_start(
                    kt_ch[d][j][:], kt[d * P:(d + 1) * P, j * MMN:(j + 1) * MMN])

            for d in range(DT):
                nc.scalar.dma_start(
                    kt_ch[d][0][:], kt[d * P:(d + 1) * P, 0:MMN])
            # exact consumption order of kb=0's matmuls: ch0 uses
            # (j0,d0),(j1,d0),(j0,d1),(j1,d1); ch1 uses (j2,d0),(j3,d0),...
            for j, d in [(0, 0), (1, 0), (0, 1), (1, 1),
                         (2, 0), (3, 0), (2, 1), (3, 1)]:
                nc.sync.dma_start(
                    qt_ch[d][j][:], qt[d * P:(d + 1) * P, j * MMN:(j + 1) * MMN])
            mbc = constp.tile([P, N], bf16, name="mbc")
            nc.sync.dma_start(mbc[:], mb[0:1, :].partition_broadcast(P))

            accA = [[psA.tile([P, MMN], f32, name=f"accA{dh}_{qc}")
                     for qc in range(2)] for dh in range(DT)]

            # Warm the PE (p-state / HAM ramp) during the initial DMA wait:
            # dummy matmuls on a zeroed tile into accA[0][0], whose garbage
            # is cleared by the first real start=True accumulation.
            zs = constp.tile([P, P], f32, name="zs")
            nc.vector.memset(zs[:], 0.0)
            zsr = zs[:].bitcast(mmdt) if mmdt != bf16 else zs[:, 0:P // 2].bitcast(bf16)
            for _ in range(16):
                nc.tensor.matmul(accA[0][0][:, 0:zsr.shape[1]], zsr, zsr,
                                 start=True, stop=True)

            e_all = [None] * KB
            w_all = [None] * KB

            def mm2(acc, kb, dh, qci):
                nc.tensor.matmul(
                    acc[:],
                    w_all[kb][:, dh * P:(dh + 1) * P],
                    e_all[kb][:, qci * MMN:(qci + 1) * MMN],
                    start=(kb == 0),
                    stop=(kb == KB - 1),
                )

            # V loads batched 4 k-blocks per DMA: one [128, 4*D] tile per
            # group, free dim laid out as (sub, d)
            v_grps = {}

            def load_vg(g):
                if g < KB // 4 and g not in v_grps:
                    v_g = vpool.tile([P, 4, D], f32, name="v_g")
                    src = v[g * 4 * P:(g + 1) * 4 * P, :].rearrange(
                        "(s p) d -> p s d", p=P)
                    nc.sync.dma_start(v_g[:], src)
                    v_grps[g] = v_g

            def v_slice(kb):
                return v_grps[kb // 4][:, kb % 4, :]

            load_vg(0)

            def mm1_exp_half(kb, ch, psS, e_kb):
                # one q-half = two 512-wide score buffers (1 PSUM bank each)
                for ch4 in (ch * 2, ch * 2 + 1):
                    st = psS.tile([P, MMN], f32, name="st")
                    for d in range(DT):
                        nc.tensor.matmul(
                            st[:],
                            kt_ch[d][kb // 4][:, (kb % 4) * P:(kb % 4 + 1) * P],
                            qt_ch[d][ch4][:],
                            start=(d == 0),
                            stop=(d == DT - 1),
                        )
                    nc.scalar.activation(
                        e_kb[:, ch4 * MMN:(ch4 + 1) * MMN], st[:],
                        EXP, scale=1.0 / 16.0)

            with tc.tile_pool(name="psS", bufs=4, space="PSUM") as psS:
                e_warm = [epool.tile([P, N], mmdt, name=f"e{kb}")
                          for kb in range(4)]
                for kb, ch in [(0, 0), (1, 0), (2, 0), (0, 1), (1, 1), (2, 1),
                               (3, 0), (3, 1)]:
                    # the q>=1024 input chunks are still in flight on the DMA
                    # bus while kb 0-2's q<1024 halves run
                    mm1_exp_half(kb, ch, psS, e_warm[kb])

                for kb in range(KB):
                    if kb % 4 == 1:
                        load_vg(kb // 4 + 1)
                    if kb in (0, 4, 8):
                        for d in range(DT):
                            load_kt(d, kb // 4 + 1)
                    if kb < 4:
                        e_kb = e_warm[kb]
                    else:
                        e_kb = epool.tile([P, N], mmdt, name=f"e{kb}")
                        for ch in range(2):
                            mm1_exp_half(kb, ch, psS, e_kb)
                    if STT_SPLIT:
                        H = N // 2
                        c_kb = cpool.tile([P, 1], f32, name="c")
                        c_lo = cpool.tile([P, 1], f32, name="c_lo")
                        nc.vector.scalar_tensor_tensor(
                            e_kb[:, 0:H], e_kb[:, 0:H], 1.0, mbc[:, 0:H],
                            MULT, MULT, accum_out=c_lo[:])
                        c_hi = cpool.tile([P, 1], f32, name="c_hi")
                        nc.vector.scalar_tensor_tensor(
                            e_kb[:, H:N], e_kb[:, H:N], 1.0, mbc[:, H:N],
                            MULT, MULT, accum_out=c_hi[:])
                        nc.vector.tensor_tensor(
                            c_kb[:], c_lo[:], c_hi[:], mybir.AluOpType.add)
                    else:
                        c_kb = cpool.tile([P, 1], f32, name="c")
                        nc.vector.scalar_tensor_tensor(
                            e_kb[:], e_kb[:], 1.0, mbc[:], MULT, MULT,
                            accum_out=c_kb[:])
                    rc = cpool.tile([P, 1], f32, name="rc")
                    nc.vector.reciprocal(rc[:], c_kb[:])
                    w_kb = wpool.tile([P, D], mmdt, name=f"w{kb}")
                    nc.vector.tensor_scalar_mul(w_kb[:], v_slice(kb), rc[:])
                    e_all[kb] = e_kb
                    w_all[kb] = w_kb

                    # interleaved half of matmul-2, LAG k-blocks behind
                    if kb >= LAG:
                        for dh in range(DT):
                            for qci in range(2):
                                mm2(accA[dh][qci], kb - LAG, dh, qci)
                for j in range(KB - LAG, KB):
                    for dh in range(DT):
                        for qci in range(2):
                            mm2(accA[dh][qci], j, dh, qci)

            # q<1024 results: copy + store (overlaps the q>=1024 matmuls)
            def store(acc, dh, qci, engine):
                o_sb = outp.tile([P, MMN], f32, name="o_sb")
                if engine == "act":
                    nc.scalar.copy(o_sb[:], acc[:])
                else:
                    nc.vector.tensor_copy(o_sb[:], acc[:])
                nc.sync.dma_start(
                    ot[dh * P:(dh + 1) * P, qci * MMN:(qci + 1) * MMN], o_sb[:])

            with tc.tile_pool(name="psB", bufs=4, space="PSUM") as psB:
                def accb_tile():
                    return psB.tile([P, MMN], f32, name="accB", tag="accB")
                for dh in range(DT):
                    for qci in range(2):
                        store(accA[dh][qci], dh, qci, "act" if dh == 0 else "dve")
                # chain-per-accumulator so each finishes early and its copy
                # overlaps the remaining accumulation chains
                for qci in range(2, NCH):
                    for dh in range(DT):
                        if (qci, dh) != (NCH - 1, DT - 1):
                            acc = accb_tile()
                            for kb in range(KB):
                                mm2(acc, kb, dh, qci)
                            store(acc, dh, qci, "act" if dh == 0 else "dve")
                        else:
                            # very last output: two half-width chains in
                            # SEPARATE banks (the second reuses the first
                            # finished chain's bank), so half A's copy+DMA
                            # fixed costs (~2.9us) hide under half B's MMs
                            o_sb = outp.tile([P, MMN], f32, name="o_sb")
                            # halves no narrower than 256: f32r matmuls drop
                            # to 1/4 rate below a 256-wide moving dim
                            for lo, W_ in ((0, 256), (256, 256)):
                                acc = accb_tile()
                                for kb in range(KB):
                                    nc.tensor.matmul(
                                        acc[:, 0:W_],
                                        w_all[kb][:, dh * P:(dh + 1) * P],
                                        e_all[kb][:, qci * MMN + lo:
                                                  qci * MMN + lo + W_],
                                        start=(kb == 0),
                                        stop=(kb == KB - 1),
                                    )
                                nc.vector.tensor_copy(o_sb[:, lo:lo + W_],
                                                      acc[:, 0:W_])
                                nc.sync.dma_start(
                                    ot[dh * P:(dh + 1) * P,
                                       qci * MMN + lo:qci * MMN + lo + W_],
                                    o_sb[:, lo:lo + W_])

    nc.compile()
    return nc


def _get_nc():
    global _cached
    if _cached is None:
        _cached = _build()
    return _cached


def kernel(key, query, value, mask):
    from concourse.bass_utils import run_bass_kernel_spmd

    nc = _get_nc()
    bf = ml_dtypes.bfloat16
    key = np.asarray(key, dtype=np.float32)
    query = np.asarray(query, dtype=np.float32)
    value = np.asarray(value, dtype=np.float32)
    mask = np.asarray(mask)

    iodt = np.float32 if DTYPE_MODE == "f32r" else bf
    in_maps = []
    for b in range(B):
        in_maps.append({
            "kt": np.ascontiguousarray(key[b].T).astype(iodt),
            "qt": np.ascontiguousarray(query[b].T).astype(iodt),
            "v": np.ascontiguousarray(value[b]),
            "mb": np.ascontiguousarray(mask[b]).astype(bf),
        })
    res = None
    for attempt in range(4):
        try:
            res = run_bass_kernel_spmd(nc, in_maps, core_ids=list(range(NCORES)))
            break
        except Exception:
            # Transient "accelerator device unrecoverable" states wedge the
            # PJRT client but not the device: tear down the backend and retry.
            if attempt == 3:
                raise
            import time
            time.sleep(10 * (attempt + 1))
            try:
                import jax.extend.backend as _jb
                _jb.clear_backends()
                import jax
                jax.clear_caches()
            except Exception:
                pass
    out = np.empty((B, N, D), np.float32)
    for b in range(B):
        out[b] = res.results[b]["ot"].T
    return out



# revision 6
# speedup vs baseline: 1.7287x; 1.7287x over previous
"""Trainium2 Bass kernel for nn_Attention_326417514823.

Per-batch computation (B=8, N=2048, D=256), one batch per NeuronCore:
    S = Q @ K.T / sqrt(D)                  (N x N)
    S[q, :] = -1e9 where mask[q] == 0      (row masking by query index)
    A = softmax(S, axis=0)                 (normalize over q, per column k)
    A[q, :] = 0 where mask[q] == 0
    O = A @ V                              (N x D)

Key restructuring vs a dense kernel:

1. HOST-SIDE QUERY COMPACTION. The softmax axis is q, and masked queries
   contribute nothing: their output rows are zero and they are excluded
   from every softmax sum. The host packs the first <=1024 unmasked
   queries into a fixed [256, 1024] device tile (pad columns are zero ->
   scores 0 -> E=1, subtracted out of the normalizer via a host-provided
   per-k correction).  Overflow queries (n_u > 1024, a ~2% tail) are
   handled exactly on the host using the device-returned normalizers
   c[k]: O_excess = (exp(S_excess)/c).T @ V.  This halves all on-device
   work (PE, exp, DMA).

2. TRANSPOSED LAYOUT. ST[k, q] = KT.T @ QT with d on partitions, so the
   softmax reduction runs along the free axis and neither matmul needs an
   on-chip transpose:
     E[k, q]  = exp(ST/16)                  (fp16, ScalarE)
     c[k]     = sum_q E[k, q] + cadj[k]     (DVE tensor_scalar accum 4x)
     W[k, :]  = V[k, :] * (1/c[k])          (fp16, DVE 4x)
     OT[d, q] = sum_k W[k, d] * E[k, q]     (PSUM accumulation over k)

3. PSUM: 2-bank [128,1024] score tiles (double-buffered, 4 banks) + all
   four [128,512] OT accumulators (4 banks) live through the whole
   k-block loop, interleaved LAG blocks behind the softmax pipeline --
   no serial phase-2 matmul tail.

Precision: bf16 Q/K (score err ~0.3%), exact exp on ACT, fp16 E/W
(~0.05%), fp32 PSUM accumulation -> rel err ~4.5e-3 (gate 2e-2).
"""

import numpy as np
import ml_dtypes

B, N, D = 8, 2048, 256
NCORES = 8
P = 128          # partitions
NU = 1024        # compacted query columns per core (device-fixed)
KB = N // P      # 16 k-blocks
NCH = NU // 512  # 2 output chunks of 512 (one PSUM bank each)
DT = D // P      # 2 d-tiles (contraction over d = 256)
LAG = 3          # k-blocks of slack before interleaved matmul-2 consumes W

_cached = None


def _build():
    import concourse.bacc as bacc
    import concourse.mybir as mybir
    import concourse.tile as tile

    f32 = mybir.dt.float32
    bf16 = mybir.dt.bfloat16
    f16 = mybir.dt.float16
    EXP = mybir.ActivationFunctionType.Exp
    ADD = mybir.AluOpType.add
    MULT = mybir.AluOpType.mult
    AX = mybir.AxisListType.X

    nc = bacc.Bacc()
    kt = nc.dram_tensor("kt", [D, N], bf16, kind="ExternalInput")
    qt = nc.dram_tensor("qt", [D, NU], bf16, kind="ExternalInput")
    v = nc.dram_tensor("v", [N, D], bf16, kind="ExternalInput")
    cadj = nc.dram_tensor("cadj", [P, KB], f32, kind="ExternalInput")
    ot = nc.dram_tensor("ot", [D, NU], f32, kind="ExternalOutput")
    cout = nc.dram_tensor("cout", [P, KB], f32, kind="ExternalOutput")

    with tile.TileContext(nc) as tc:
        with (
            tc.tile_pool(name="const", bufs=1) as constp,
            tc.tile_pool(name="epool", bufs=1) as epool,
            tc.tile_pool(name="cpool", bufs=1) as cpool,
            tc.tile_pool(name="outp", bufs=4) as outp,
            # all 4 OT accumulators live for the whole kernel (banks 0-3)
            tc.tile_pool(name="psA", bufs=1, space="PSUM") as psA,
        ):
            # SBUF inputs. kt/qt land as [d_part, d_tile, cols] so each
            # matmul slices a [128, x] 2D AP. DMA split so kb0 can start
            # after ~0.4MB; emission order == first-consumption order.
            kt_sb = constp.tile([P, DT, N], bf16, name="kt_sb")
            qt_sb = constp.tile([P, DT, NU], bf16, name="qt_sb")
            v_sb = constp.tile([P, KB, D], bf16, name="v_sb")
            w_sb = constp.tile([P, KB, D], f16, name="w_sb")
            cadj_sb = constp.tile([P, KB], f32, name="cadj_sb")
            ctile = cpool.tile([P, KB], f32, name="ctile")
            rctile = cpool.tile([P, KB], f32, name="rctile")

            def dram_dtiled(t, cols0, cols1):
                # [D, x] DRAM slice -> [128, DT, x] (d_part, d_tile, col)
                return t[:, cols0:cols1].rearrange("(t p) c -> p t c", p=P)

            # kb0 needs kt cols 0:128 and ALL qt; interleave so the first
            # matmul's operands arrive first.
            nc.scalar.dma_start(kt_sb[:, :, 0:512], dram_dtiled(kt, 0, 512))
            nc.sync.dma_start(qt_sb[:, :, 0:512], dram_dtiled(qt, 0, 512))
            nc.sync.dma_start(qt_sb[:, :, 512:NU], dram_dtiled(qt, 512, NU))
            nc.scalar.dma_start(cadj_sb[:], cadj[:, :])
            nc.sync.dma_start(kt_sb[:, :, 512:N], dram_dtiled(kt, 512, N))
            for g in range(4):
                nc.scalar.dma_start(
                    v_sb[:, 4 * g:4 * (g + 1), :],
                    v[4 * g * P:4 * (g + 1) * P, :].rearrange(
                        "(s p) d -> p s d", p=P))

            accA = [[psA.tile([P, 512], f32, name=f"accA{dh}_{ch}")
                     for ch in range(NCH)] for dh in range(DT)]

            # Warm the PE (p-state ramp) during the initial DMA wait; the
            # garbage lands in accA[0][0] and is cleared by its first
            # start=True accumulation.
            zs = constp.tile([P, P], bf16, name="zs")
            nc.vector.memset(zs[:], 0.0)
            for _ in range(20):
                nc.tensor.matmul(accA[0][0][:, 0:P], zs[:], zs[:],
                                 start=True, stop=True)

            e_all = [None] * KB

            def mm2(kb):
                for dh in range(DT):
                    for ch in range(NCH):
                        nc.tensor.matmul(
                            accA[dh][ch][:],
                            w_sb[:, kb, dh * P:(dh + 1) * P],
                            e_all[kb][:, ch * 512:(ch + 1) * 512],
                            start=(kb == 0),
                            stop=(kb == KB - 1),
                        )

            with tc.tile_pool(name="psS", bufs=2, space="PSUM") as psS:
                for kb in range(KB):
                    st = psS.tile([P, NU], f32, name="st")
                    for ch in range(NCH):
                        for dt in range(DT):
                            nc.tensor.matmul(
                                st[:, ch * 512:(ch + 1) * 512],
                                kt_sb[:, dt, kb * P:(kb + 1) * P],
                                qt_sb[:, dt, ch * 512:(ch + 1) * 512],
                                start=(dt == 0),
                                stop=(dt == DT - 1),
                            )
                    e_kb = epool.tile([P, NU], f16, name=f"e{kb}")
                    # c[k] = sum_q E, accumulated by the ACT engine during exp
                    nc.scalar.activation(e_kb[:], st[:], EXP, scale=1.0 / 16.0,
                                         accum_out=ctile[:, kb:kb + 1])
                    nc.vector.tensor_tensor(
                        ctile[:, kb:kb + 1], ctile[:, kb:kb + 1],
                        cadj_sb[:, kb:kb + 1], ADD)
                    nc.vector.reciprocal(rctile[:, kb:kb + 1],
                                         ctile[:, kb:kb + 1])
                    nc.vector.tensor_scalar_mul(
                        w_sb[:, kb, :], v_sb[:, kb, :], rctile[:, kb:kb + 1])
                    e_all[kb] = e_kb
                    if kb >= LAG:
                        mm2(kb - LAG)
                for kb in range(KB - LAG, KB):
                    mm2(kb)

            nc.sync.dma_start(cout[:, :], ctile[:])
            for dh in range(DT):
                for ch in range(NCH):
                    o_sb = outp.tile([P, 512], f32, name="o_sb")
                    if dh == 0:
                        nc.scalar.copy(o_sb[:], accA[dh][ch][:])
                    else:
                        nc.vector.tensor_copy(o_sb[:], accA[dh][ch][:])
                    nc.sync.dma_start(
                        ot[dh * P:(dh + 1) * P, ch * 512:(ch + 1) * 512],
                        o_sb[:])

    nc.compile()
    return nc


def _get_nc():
    global _cached
    if _cached is None:
        _cached = _build()
    return _cached


def kernel(key, query, value, mask):
    from concourse.bass_utils import run_bass_kernel_spmd

    nc = _get_nc()
    bf = ml_dtypes.bfloat16
    key = np.asarray(key, dtype=np.float32)
    query = np.asarray(query, dtype=np.float32)
    value = np.asarray(value, dtype=np.float32)
    mask = np.asarray(mask)

    in_maps = []
    host = []  # per-batch host-side state for postprocessing
    for b in range(B):
        m = mask[b, 0].astype(bool)
        idx = np.nonzero(m)[0]
        dev_idx = idx[:NU]
        ex_idx = idx[NU:]
        nd = len(dev_idx)
        npad = NU - nd

        qdev = np.zeros((NU, D), np.float32)
        qdev[:nd] = query[b][dev_idx]
        # cextra[k]: contribution of host-handled overflow queries to the
        # softmax normalizer; computed with the same exp(S/16) the device
        # uses. Pad columns contribute exp(0)=1 each -> subtract npad.
        if len(ex_idx):
            s_ex = (key[b] @ query[b][ex_idx].T) / 16.0   # (N, ne)
            e_ex = np.exp(s_ex)
            cextra = e_ex.sum(axis=1)
        else:
            e_ex = None
            cextra = np.zeros(N, np.float32)
        cadj = (cextra - float(npad)).astype(np.float32)

        in_maps.append({
            "kt": np.ascontiguousarray(key[b].T).astype(bf),
            "qt": np.ascontiguousarray(qdev.T).astype(bf),
            "v": np.ascontiguousarray(value[b]).astype(bf),
            "cadj": np.ascontiguousarray(cadj.reshape(KB, P).T),
        })
        host.append((dev_idx, ex_idx, nd, e_ex))

    res = None
    for attempt in range(4):
        try:
            res = run_bass_kernel_spmd(nc, in_maps, core_ids=list(range(NCORES)))
            break
        except Exception:
            # Transient "accelerator device unrecoverable" states wedge the
            # PJRT client but not the device: tear down the backend and retry.
            if attempt == 3:
                raise
            import time
            time.sleep(10 * (attempt + 1))
            try:
                import jax.extend.backend as _jb
                _jb.clear_backends()
                import jax
                jax.clear_caches()
            except Exception:
                pass

    out = np.zeros((B, N, D), np.float32)
    for b in range(B):
        dev_idx, ex_idx, nd, e_ex = host[b]
        otb = res.results[b]["ot"]          # (D, NU)
        out[b][dev_idx] = otb.T[:nd]
        if len(ex_idx):
            c = res.results[b]["cout"].T.reshape(N)   # (N,) corrected c
            a_ex = e_ex / c[:, None]                  # (N, ne)
            out[b][ex_idx] = a_ex.T @ value[b]
    return out


# revision 7
# speedup vs baseline: 1.8748x; 1.0845x over previous
"""Trainium2 Bass kernel for nn_Attention_326417514823.

Per-batch computation (B=8, N=2048, D=256), one batch per NeuronCore:
    S = Q @ K.T / sqrt(D)                  (N x N)
    S[q, :] = -1e9 where mask[q] == 0      (row masking by query index)
    A = softmax(S, axis=0)                 (normalize over q, per column k)
    A[q, :] = 0 where mask[q] == 0
    O = A @ V                              (N x D)

Key restructuring vs a dense kernel:

1. HOST-SIDE QUERY COMPACTION. The softmax axis is q, and masked queries
   contribute nothing: their output rows are zero and they are excluded
   from every softmax sum. The host packs the first <=1024 unmasked
   queries into a fixed [256, 1024] device tile (pad columns are zero ->
   scores 0 -> E=1, subtracted out of the normalizer via a host-provided
   per-k correction).  Overflow queries (n_u > 1024, a ~2% tail) are
   handled exactly on the host using the device-returned normalizers
   c[k]: O_excess = (exp(S_excess)/c).T @ V.  This halves all on-device
   work (PE, exp, DMA).

2. TRANSPOSED LAYOUT. ST[k, q] = KT.T @ QT with d on partitions, so the
   softmax reduction runs along the free axis and neither matmul needs an
   on-chip transpose:
     E[k, q]  = exp(ST/16)                   (fp16, ScalarE, fused accum c)
     W[k, :]  = V[k, :] * (1/c[k])           (fp16, DVE)
     OT[d, q] = sum_k W[k, d] * E[k, q]      (PSUM accumulation over k)

3. MM1 runs as 3 fp8e4 DoubleRow matmuls (hi/lo error compensation:
   Kh Qh + Kh Ql + Kl Qh), 0.5 cycles/row with a 256-deep contraction:
   25% fewer PE cycles than one bf16 pass, ~0.6% score error.

4. PSUM: 2-bank [128,1024] score tiles (double-buffered, 4 banks) + all
   four [128,512] OT accumulators (4 banks) live through the whole
   k-block loop, LAG blocks behind the softmax pipeline -- no serial
   matmul phase-2. Chains drain one at a time at the end so each copy
   (DVE, ->fp16) + store overlaps the next chain's matmuls.

Precision: fp8-hilo scores (~0.6%), exact exp on ACT, fp16 E/W, fp32
PSUM accumulation, fp16 output staging -> rel err ~4.7e-3 (gate 2e-2).
"""

import numpy as np
import ml_dtypes

B, N, D = 8, 2048, 256
NCORES = 8
P = 128          # partitions
NU = 1024        # compacted query columns per core (device-fixed)
KB = N // P      # 16 k-blocks
NCH = NU // 512  # 2 output chunks of 512 (one PSUM bank each)
DT = D // P      # 2 d-tiles (contraction over d = 256)
LAG = 3          # k-blocks of slack before interleaved matmul-2 consumes W

_cached = None


def _build():
    import concourse.bacc as bacc
    import concourse.mybir as mybir
    import concourse.tile as tile

    f32 = mybir.dt.float32
    bf16 = mybir.dt.bfloat16
    f16 = mybir.dt.float16
    f8 = mybir.dt.float8e4
    DR = mybir.MatmulPerfMode.DoubleRow
    EXP = mybir.ActivationFunctionType.Exp
    ADD = mybir.AluOpType.add

    nc = bacc.Bacc()
    # hi/lo fp8 pairs, concatenated on a leading axis: [2(hl), D, cols]
    kt8 = nc.dram_tensor("kt8", [2, D, N], f8, kind="ExternalInput")
    qt8 = nc.dram_tensor("qt8", [2, D, NU], f8, kind="ExternalInput")
    v = nc.dram_tensor("v", [N, D], bf16, kind="ExternalInput")
    cadj = nc.dram_tensor("cadj", [P, KB], f32, kind="ExternalInput")
    ot = nc.dram_tensor("ot", [D, NU], f16, kind="ExternalOutput")
    cout = nc.dram_tensor("cout", [P, KB], f32, kind="ExternalOutput")

    with tile.TileContext(nc) as tc:
        with (
            tc.tile_pool(name="const", bufs=1) as constp,
            tc.tile_pool(name="epool", bufs=1) as epool,
            tc.tile_pool(name="cpool", bufs=1) as cpool,
            tc.tile_pool(name="outp", bufs=4) as outp,
            # all 4 OT accumulators live for the whole kernel (banks 0-3)
            tc.tile_pool(name="psA", bufs=1, space="PSUM") as psA,
        ):
            # SBUF inputs: [d_part, hl, d_tile, cols] so each DoubleRow
            # matmul slices a [128, 2, x] 3D AP (contraction d = part+tile).
            kt_sb = constp.tile([P, 2, DT, N], f8, name="kt_sb")
            qt_sb = constp.tile([P, 2, DT, NU], f8, name="qt_sb")
            v_sb = constp.tile([P, KB, D], bf16, name="v_sb")
            w_sb = constp.tile([P, KB, D], f16, name="w_sb")
            cadj_sb = constp.tile([P, KB], f32, name="cadj_sb")
            ctile = cpool.tile([P, KB], f32, name="ctile")
            rctile = cpool.tile([P, KB], f32, name="rctile")

            def dram_hl(t, cols0, cols1):
                # [2, D, x] DRAM slice -> [128, 2(hl), DT, x]
                return t[:, :, cols0:cols1].rearrange(
                    "h (t p) c -> p h t c", p=P)

            # kb0/ch0 operands first; the rest streams behind.
            nc.scalar.dma_start(kt_sb[:, :, :, 0:512], dram_hl(kt8, 0, 512))
            nc.sync.dma_start(qt_sb[:, :, :, 0:512], dram_hl(qt8, 0, 512))
            nc.sync.dma_start(qt_sb[:, :, :, 512:NU], dram_hl(qt8, 512, NU))
            nc.scalar.dma_start(cadj_sb[:], cadj[:, :])
            nc.sync.dma_start(kt_sb[:, :, :, 512:N], dram_hl(kt8, 512, N))
            for g in range(2):
                nc.scalar.dma_start(
                    v_sb[:, 8 * g:8 * (g + 1), :],
                    v[8 * g * P:8 * (g + 1) * P, :].rearrange(
                        "(s p) d -> p s d", p=P))

            accA = [[psA.tile([P, 512], f32, name=f"accA{dh}_{ch}")
                     for ch in range(NCH)] for dh in range(DT)]

            # Warm the PE (p-state ramp) during the initial DMA wait; the
            # garbage lands in accA[0][0] and is cleared by its first
            # start=True accumulation.
            zs = constp.tile([P, 256], f8, name="zs")
            nc.vector.memset(zs[:], 0.0)
            for _ in range(14):
                nc.tensor.matmul(accA[0][0][:, 0:256], zs[:, 0:P],
                                 zs[:], start=True, stop=True)

            e_all = [None] * KB

            def mm2_step(dh, ch, kb):
                nc.tensor.matmul(
                    accA[dh][ch][:],
                    w_sb[:, kb, dh * P:(dh + 1) * P],
                    e_all[kb][:, ch * 512:(ch + 1) * 512],
                    start=(kb == 0),
                    stop=(kb == KB - 1),
                )

            with tc.tile_pool(name="psS", bufs=2, space="PSUM") as psS:
                for kb in range(KB):
                    st = psS.tile([P, NU], f32, name="st")
                    kw = (slice(None), slice(None))  # placeholder
                    for ch in range(NCH):
                        cs = slice(ch * 512, (ch + 1) * 512)
                        ks = slice(kb * P, (kb + 1) * P)
                        # hi*hi, hi*lo, lo*hi fp8 DoubleRow accumulation
                        for i, (hk, hq) in enumerate(((0, 0), (0, 1), (1, 0))):
                            nc.tensor.matmul(
                                st[:, cs],
                                kt_sb[:, hk, :, ks],
                                qt_sb[:, hq, :, cs],
                                start=(i == 0),
                                stop=(i == 2),
                                perf_mode=DR,
                            )
                    e_kb = epool.tile([P, NU], f16, name=f"e{kb}")
                    # c[k] = sum_q E, accumulated by the ACT engine during exp
                    nc.scalar.activation(e_kb[:], st[:], EXP, scale=1.0 / 16.0,
                                         accum_out=ctile[:, kb:kb + 1])
                    nc.vector.tensor_tensor(
                        ctile[:, kb:kb + 1], ctile[:, kb:kb + 1],
                        cadj_sb[:, kb:kb + 1], ADD)
                    nc.vector.reciprocal(rctile[:, kb:kb + 1],
                                         ctile[:, kb:kb + 1])
                    nc.vector.tensor_scalar_mul(
                        w_sb[:, kb, :], v_sb[:, kb, :], rctile[:, kb:kb + 1])
                    e_all[kb] = e_kb
                    if kb >= LAG:
                        for dh in range(DT):
                            for ch in range(NCH):
                                mm2_step(dh, ch, kb - LAG)

                nc.sync.dma_start(cout[:, :], ctile[:])
                # Drain chain-by-chain: each chain's copy+store overlaps the
                # next chain's remaining matmuls.
                for dh in range(DT):
                    for ch in range(NCH):
                        for kb in range(KB - LAG, KB):
                            mm2_step(dh, ch, kb)
                        o_sb = outp.tile([P, 512], f16, name="o_sb")
                        nc.vector.tensor_copy(o_sb[:], accA[dh][ch][:])
                        nc.sync.dma_start(
                            ot[dh * P:(dh + 1) * P, ch * 512:(ch + 1) * 512],
                            o_sb[:])

    nc.compile()
    return nc


def _get_nc():
    global _cached
    if _cached is None:
        _cached = _build()
    return _cached


def _hilo8(x):
    """fp8e4m3 hi/lo decomposition along a new leading axis."""
    f8n = ml_dtypes.float8_e4m3
    hi = x.astype(f8n)
    lo = (x - hi.astype(np.float32)).astype(f8n)
    return np.stack([hi, lo], axis=0)


def kernel(key, query, value, mask):
    from concourse.bass_utils import run_bass_kernel_spmd

    nc = _get_nc()
    bf = ml_dtypes.bfloat16
    key = np.asarray(key, dtype=np.float32)
    query = np.asarray(query, dtype=np.float32)
    value = np.asarray(value, dtype=np.float32)
    mask = np.asarray(mask)

    in_maps = []
    host = []  # per-batch host-side state for postprocessing
    for b in range(B):
        m = mask[b, 0].astype(bool)
        idx = np.nonzero(m)[0]
        dev_idx = idx[:NU]
        ex_idx = idx[NU:]
        nd = len(dev_idx)
        npad = NU - nd

        qdev = np.zeros((NU, D), np.float32)
        qdev[:nd] = query[b][dev_idx]
        # cextra[k]: contribution of host-handled overflow queries to the
        # softmax normalizer. Pad columns contribute exp(0)=1 each.
        if len(ex_idx):
            s_ex = (key[b] @ query[b][ex_idx].T) / 16.0   # (N, ne)
            e_ex = np.exp(s_ex)
            cextra = e_ex.sum(axis=1)
        else:
            e_ex = None
            cextra = np.zeros(N, np.float32)
        cadj = (cextra - float(npad)).astype(np.float32)

        in_maps.append({
            "kt8": _hilo8(np.ascontiguousarray(key[b].T)),
            "qt8": _hilo8(np.ascontiguousarray(qdev.T)),
            "v": np.ascontiguousarray(value[b]).astype(bf),
            "cadj": np.ascontiguousarray(cadj.reshape(KB, P).T),
        })
        host.append((dev_idx, ex_idx, nd, e_ex))

    res = None
    for attempt in range(4):
        try:
            res = run_bass_kernel_spmd(nc, in_maps, core_ids=list(range(NCORES)))
            break
        except Exception:
            # Transient "accelerator device unrecoverable" states wedge the
            # PJRT client but not the device: tear down the backend and retry.
            if attempt == 3:
                raise
            import time
            time.sleep(10 * (attempt + 1))
            try:
                import jax.extend.backend as _jb
                _jb.clear_backends()
                import jax
                jax.clear_caches()
            except Exception:
                pass

    out = np.zeros((B, N, D), np.float32)
    for b in range(B):
        dev_idx, ex_idx, nd, e_ex = host[b]
        otb = res.results[b]["ot"].astype(np.float32)   # (D, NU)
        out[b][dev_idx] = otb.T[:nd]
        if len(ex_idx):
            c = res.results[b]["cout"].T.reshape(N)     # (N,) corrected c
            a_ex = e_ex / c[:, None]                    # (N, ne)
            out[b][ex_idx] = a_ex.T @ value[b]
    return out


# revision 10
# speedup vs baseline: 1.8933x; 1.0099x over previous
"""Trainium2 Bass kernel for nn_Attention_326417514823.

Per-batch computation (B=8, N=2048, D=256), one batch per NeuronCore:
    S = Q @ K.T / sqrt(D)                  (N x N)
    S[q, :] = -1e9 where mask[q] == 0      (row masking by query index)
    A = softmax(S, axis=0)                 (normalize over q, per column k)
    A[q, :] = 0 where mask[q] == 0
    O = A @ V                              (N x D)

Key restructuring vs a dense kernel:

1. HOST-SIDE QUERY COMPACTION. The softmax axis is q, and masked queries
   contribute nothing: their output rows are zero and they are excluded
   from every softmax sum. The host packs the first <=1024 unmasked
   queries into a fixed [256, 1024] device tile (pad columns are zero ->
   scores 0 -> E=1, subtracted out of the normalizer via a host-provided
   per-k correction).  Overflow queries (n_u > 1024, a ~2% tail) are
   handled exactly on the host using the device-returned normalizers
   c[k]: O_excess = (exp(S_excess)/c).T @ V.  This halves all on-device
   work (PE, exp, DMA).

2. TRANSPOSED LAYOUT. ST[k, q] = KT.T @ QT with d on partitions, so the
   softmax reduction runs along the free axis and neither matmul needs an
   on-chip transpose:
     E[k, q]  = exp(ST/16)                   (fp16, ScalarE, fused accum c)
     W[k, :]  = V[k, :] * (1/c[k])           (fp16, DVE)
     OT[d, q] = sum_k W[k, d] * E[k, q]      (PSUM accumulation over k)

3. MM1 runs as 3 fp8e4 DoubleRow matmuls (hi/lo error compensation:
   Kh Qh + Kh Ql + Kl Qh), 0.5 cycles/row with a 256-deep contraction:
   25% fewer PE cycles than one bf16 pass, ~0.6% score error.

4. PSUM: 2-bank [128,1024] score tiles (double-buffered, 4 banks) + all
   four [128,512] OT accumulators (4 banks) live through the whole
   k-block loop, LAG blocks behind the softmax pipeline -- no serial
   matmul phase-2. Chains drain one at a time at the end so each copy
   (DVE, ->fp16) + store overlaps the next chain's matmuls.

Precision: fp8-hilo scores (~0.6%), exact exp on ACT, fp16 E/W, fp32
PSUM accumulation, fp16 output staging -> rel err ~4.7e-3 (gate 2e-2).
"""

import numpy as np
import ml_dtypes

B, N, D = 8, 2048, 256
NCORES = 8
P = 128          # partitions
NU = 1024        # compacted query columns per core (device-fixed)
KB = N // P      # 16 k-blocks
NCH = NU // 512  # 2 output chunks of 512 (one PSUM bank each)
DT = D // P      # 2 d-tiles (contraction over d = 256)
LAG = 3          # k-blocks of slack before interleaved matmul-2 consumes W

_cached = None


def _build():
    import concourse.bacc as bacc
    import concourse.mybir as mybir
    import concourse.tile as tile

    f32 = mybir.dt.float32
    bf16 = mybir.dt.bfloat16
    f16 = mybir.dt.float16
    f8 = mybir.dt.float8e4
    DR = mybir.MatmulPerfMode.DoubleRow
    EXP = mybir.ActivationFunctionType.Exp
    ADD = mybir.AluOpType.add

    nc = bacc.Bacc()
    # hi/lo fp8 pairs, concatenated on a leading axis: [2(hl), D, cols]
    kt8 = nc.dram_tensor("kt8", [2, D, N], f8, kind="ExternalInput")
    qt8 = nc.dram_tensor("qt8", [2, D, NU], f8, kind="ExternalInput")
    v = nc.dram_tensor("v", [N, D], bf16, kind="ExternalInput")
    cadj = nc.dram_tensor("cadj", [P, KB], f32, kind="ExternalInput")
    ot = nc.dram_tensor("ot", [D, NU], f16, kind="ExternalOutput")
    cout = nc.dram_tensor("cout", [P, KB], f32, kind="ExternalOutput")

    with tile.TileContext(nc) as tc:
        with (
            tc.tile_pool(name="const", bufs=1) as constp,
            tc.tile_pool(name="epool", bufs=1) as epool,
            tc.tile_pool(name="cpool", bufs=1) as cpool,
            tc.tile_pool(name="outp", bufs=4) as outp,
            # all 4 OT accumulators live for the whole kernel (banks 0-3)
            tc.tile_pool(name="psA", bufs=1, space="PSUM") as psA,
        ):
            # SBUF inputs: [d_part, hl, d_tile, cols] so each DoubleRow
            # matmul slices a [128, 2, x] 3D AP (contraction d = part+tile).
            kt_sb = constp.tile([P, 2, DT, N], f8, name="kt_sb")
            qt_sb = constp.tile([P, 2, DT, NU], f8, name="qt_sb")
            v_sb = constp.tile([P, KB, D], bf16, name="v_sb")
            w_sb = constp.tile([P, KB, D], f16, name="w_sb")
            cadj_sb = constp.tile([P, KB], f32, name="cadj_sb")
            ctile = cpool.tile([P, KB], f32, name="ctile")
            rctile = cpool.tile([P, KB], f32, name="rctile")

            def dram_hl(t, cols0, cols1):
                # [2, D, x] DRAM slice -> [128, 2(hl), DT, x]
                return t[:, :, cols0:cols1].rearrange(
                    "h (t p) c -> p h t c", p=P)

            # Ordered by first consumption: kb0 needs kt cols 0:128 + all qt;
            # kb2+ needs later kt chunks; v at the k-block pace; cadj by kb0's
            # c-correction (~5us in).
            nc.scalar.dma_start(kt_sb[:, :, :, 0:256], dram_hl(kt8, 0, 256))
            nc.sync.dma_start(qt_sb[:, :, :, 0:512], dram_hl(qt8, 0, 512))
            nc.sync.dma_start(qt_sb[:, :, :, 512:NU], dram_hl(qt8, 512, NU))
            nc.scalar.dma_start(kt_sb[:, :, :, 256:1024],
                                dram_hl(kt8, 256, 1024))
            nc.sync.dma_start(kt_sb[:, :, :, 1024:N], dram_hl(kt8, 1024, N))
            nc.scalar.dma_start(
                v_sb[:, 0:8, :],
                v[0:8 * P, :].rearrange("(s p) d -> p s d", p=P))
            nc.sync.dma_start(cadj_sb[:], cadj[:, :])
            nc.scalar.dma_start(
                v_sb[:, 8:KB, :],
                v[8 * P:KB * P, :].rearrange("(s p) d -> p s d", p=P))

            accA = [[psA.tile([P, 512], f32, name=f"accA{dh}_{ch}")
                     for ch in range(NCH)] for dh in range(DT)]

            # Warm the PE (p-state ramp) during the initial DMA wait; the
            # garbage lands in accA[0][0] and is cleared by its first
            # start=True accumulation.
            zs = constp.tile([P, 256], f8, name="zs")
            nc.vector.memset(zs[:], 0.0)
            for _ in range(11):
                nc.tensor.matmul(accA[0][0][:, 0:256], zs[:, 0:P],
                                 zs[:], start=True, stop=True)

            e_all = [None] * KB

            def mm2_step(dh, ch, kb):
                nc.tensor.matmul(
                    accA[dh][ch][:],
                    w_sb[:, kb, dh * P:(dh + 1) * P],
                    e_all[kb][:, ch * 512:(ch + 1) * 512],
                    start=(kb == 0),
                    stop=(kb == KB - 1),
                )

            with tc.tile_pool(name="psS", bufs=2, space="PSUM") as psS:
                for kb in range(KB):
                    st = psS.tile([P, NU], f32, name="st")
                    kw = (slice(None), slice(None))  # placeholder
                    for ch in range(NCH):
                        cs = slice(ch * 512, (ch + 1) * 512)
                        ks = slice(kb * P, (kb + 1) * P)
                        # hi*hi, hi*lo, lo*hi fp8 DoubleRow accumulation
                        for i, (hk, hq) in enumerate(((0, 0), (0, 1), (1, 0))):
                            nc.tensor.matmul(
                                st[:, cs],
                                kt_sb[:, hk, :, ks],
                                qt_sb[:, hq, :, cs],
                                start=(i == 0),
                                stop=(i == 2),
                                perf_mode=DR,
                            )
                    e_kb = epool.tile([P, NU], f16, name=f"e{kb}")
                    # c[k] = sum_q E, accumulated by the ACT engine during exp
                    nc.scalar.activation(e_kb[:], st[:], EXP, scale=1.0 / 16.0,
                                         accum_out=ctile[:, kb:kb + 1])
                    nc.vector.tensor_tensor(
                        ctile[:, kb:kb + 1], ctile[:, kb:kb + 1],
                        cadj_sb[:, kb:kb + 1], ADD)
                    nc.vector.reciprocal(rctile[:, kb:kb + 1],
                                         ctile[:, kb:kb + 1])
                    nc.vector.tensor_scalar_mul(
                        w_sb[:, kb, :], v_sb[:, kb, :], rctile[:, kb:kb + 1])
                    e_all[kb] = e_kb
                    if kb >= LAG:
                        for dh in range(DT):
                            for ch in range(NCH):
                                mm2_step(dh, ch, kb - LAG)

                nc.sync.dma_start(cout[:, :], ctile[:])
                # Drain chain-by-chain; copies split across DVE and ACT so
                # two run in parallel while later chains finish on the PE.
                for i, (dh, ch) in enumerate([(dh, ch) for dh in range(DT)
                                              for ch in range(NCH)]):
                    for kb in range(KB - LAG, KB):
                        mm2_step(dh, ch, kb)
                    o_sb = outp.tile([P, 512], f16, name="o_sb")
                    if i % 2 == 0:
                        nc.vector.tensor_copy(o_sb[:], accA[dh][ch][:])
                    else:
                        nc.scalar.copy(o_sb[:], accA[dh][ch][:])
                    nc.sync.dma_start(
                        ot[dh * P:(dh + 1) * P, ch * 512:(ch + 1) * 512],
                        o_sb[:])

    nc.compile()
    return nc


def _get_nc():
    global _cached
    if _cached is None:
        _cached = _build()
    return _cached


def _hilo8(x):
    """fp8e4m3 hi/lo decomposition along a new leading axis."""
    f8n = ml_dtypes.float8_e4m3
    hi = x.astype(f8n)
    lo = (x - hi.astype(np.float32)).astype(f8n)
    return np.stack([hi, lo], axis=0)


def kernel(key, query, value, mask):
    from concourse.bass_utils import run_bass_kernel_spmd

    nc = _get_nc()
    bf = ml_dtypes.bfloat16
    key = np.asarray(key, dtype=np.float32)
    query = np.asarray(query, dtype=np.float32)
    value = np.asarray(value, dtype=np.float32)
    mask = np.asarray(mask)

    in_maps = []
    host = []  # per-batch host-side state for postprocessing
    for b in range(B):
        m = mask[b, 0].astype(bool)
        idx = np.nonzero(m)[0]
        dev_idx = idx[:NU]
        ex_idx = idx[NU:]
        nd = len(dev_idx)
        npad = NU - nd

        qdev = np.zeros((NU, D), np.float32)
        qdev[:nd] = query[b][dev_idx]
        # cextra[k]: contribution of host-handled overflow queries to the
        # softmax normalizer. Pad columns contribute exp(0)=1 each.
        if len(ex_idx):
            s_ex = (key[b] @ query[b][ex_idx].T) / 16.0   # (N, ne)
            e_ex = np.exp(s_ex)
            cextra = e_ex.sum(axis=1)
        else:
            e_ex = None
            cextra = np.zeros(N, np.float32)
        cadj = (cextra - float(npad)).astype(np.float32)

        in_maps.append({
            "kt8": _hilo8(np.ascontiguousarray(key[b].T)),
            "qt8": _hilo8(np.ascontiguousarray(qdev.T)),
            "v": np.ascontiguousarray(value[b]).astype(bf),
            "cadj": np.ascontiguousarray(cadj.reshape(KB, P).T),
        })
        host.append((dev_idx, ex_idx, nd, e_ex))

    res = None
    for attempt in range(4):
        try:
            res = run_bass_kernel_spmd(nc, in_maps, core_ids=list(range(NCORES)))
            break
        except Exception:
            # Transient "accelerator device unrecoverable" states wedge the
            # PJRT client but not the device: tear down the backend and retry.
            if attempt == 3:
                raise
            import time
            time.sleep(10 * (attempt + 1))
            try:
                import jax.extend.backend as _jb
                _jb.clear_backends()
                import jax
                jax.clear_caches()
            except Exception:
                pass

    out = np.zeros((B, N, D), np.float32)
    for b in range(B):
        dev_idx, ex_idx, nd, e_ex = host[b]
        otb = res.results[b]["ot"].astype(np.float32)   # (D, NU)
        out[b][dev_idx] = otb.T[:nd]
        if len(ex_idx):
            c = res.results[b]["cout"].T.reshape(N)     # (N,) corrected c
            a_ex = e_ex / c[:, None]                    # (N, ne)
            out[b][ex_idx] = a_ex.T @ value[b]
    return out
